# revision 25
# baseline (speedup 1.0000x reference)
"""Trainium2 Bass kernel for nn_Attention_6932077215914 (GQA attention layer).

Strategy (8 NeuronCores, tensor-parallel over heads + sequence-parallel dense):
  - Host prep: x -> x^T (bf16), softmax scale folded into w_q, w_q/w_kv sharded
    by head/KV-group, weights pre-tiled to SBUF layout. bf16 compute, fp32 PSUM.
  - Core c owns heads {2c, 2c+1} (KV group c//2). Within each core pair the KV
    projection is split (even cores K^T, odd cores V^T; split lives in the
    DATA so the graph stays SPMD-uniform); a per-batch 2-rank AllGather
    exchanges the halves.
  - v2 restructure (from perfetto: PE busy 275us of 335us span, phase gaps +
    HAM-cold windows at phase seams, 16us startup, 12us tail):
    * One dense PE stream: b=1 projection matmul groups are zipped INTO
      attention chunk (0,0) as fill work; dense b=0 groups zip into chunks
      (1,0)/(1,1); dense b=1 is split into hl-halves so its hl=0 half plus
      deferred b=0 groups cover the last AllToAll drain.
    * Tiny dummy pair-AllGather fired at t~7us flushes the runtime's CC init
      barrier before the real K/V AllGather needs it.
    * x^T staged through a rotating per-(sc,hc) tile pool (6MB instead of
      16MB resident) so attention slabs can overlap the projection tail.
    * PSUM repacked to 8 banks: acc tag (proj + dense-b0, 2) + score slabs
      (2x[128,1024], 4) + ctx (2); fold/broadcast outputs share the score
      rotation; dense-b1 halves get a 6-deep rotation after attention PSUM
      closes.
  - AllToAll per (batch, local-head) chunk redistributes ctx^T so each core
    owns a 256-token slice for the dense projection.
"""

import sys
import types

import numpy as np
import ml_dtypes

B, SQ, HIDDEN = 2, 2048, 2048
HEADS, GROUPS, KVC = 16, 4, 128
SCALE = KVC ** -0.5
NCORES = 8
T = B * SQ               # 4096 flattened tokens
TC = 512                 # t-chunk for QKV projection
NTC = T // TC            # 8


def _install_ntff_hook():
    """boot() skips NTFF hook registration when the image's antenv lacks
    axon_hooks; recreate the tiny module so trace=True / BASS_TRACE works."""
    if "antenv.axon_hooks" in sys.modules:
        return
    try:
        from trn_agent_boot.trn_boot import _ntff_profile_via_ctypes
        hook = _ntff_profile_via_ctypes("/opt/axon/libaxon_pjrt.so")
    except Exception:
        return
    mod = types.ModuleType("antenv.axon_hooks")
    mod.get_axon_ntff_profile_hook = lambda: hook
    mod.set_axon_ntff_profile_hook = lambda h: None
    sys.modules["antenv.axon_hooks"] = mod


_install_ntff_hook()

_CACHE = {}


def _build():
    import contextlib

    import concourse.bass as bass
    import concourse.mybir as mybir
    import concourse.tile as tile
    from concourse import bacc
    from concourse.bass import ts, ds

    BF16 = mybir.dt.bfloat16
    F32 = mybir.dt.float32
    AF = mybir.ActivationFunctionType

    nc = bacc.Bacc("TRN2", target_bir_lowering=False, debug=False,
                   num_devices=NCORES)

    # x pre-transposed host-side to partition-major [p, sc, hc, t] so each
    # 512-token chunk is ONE fully-contiguous 2MB DMA (a strided gather
    # from [HIDDEN, T] runs at ~half DMA rate and starved the start)
    xt = nc.dram_tensor("xt", [128, NTC * 16 * TC], BF16,
                        kind="ExternalInput")
    wq = nc.dram_tensor("wq", [128, 16 * 256], BF16, kind="ExternalInput")
    # per-core HALF of the kv projection: even cores w_k, odd cores w_v
    wkv = nc.dram_tensor("wkv", [128, 16 * 128], BF16, kind="ExternalInput")
    wd = nc.dram_tensor("wd", [128, 16 * HIDDEN], BF16, kind="ExternalInput")
    out = nc.dram_tensor("out", [512, HIDDEN], F32, kind="ExternalOutput")

    PAIRS = [[2 * i, 2 * i + 1] for i in range(NCORES // 2)]

    def off_of(kt, qg):
        r = kt - 4 * qg
        return 128 * r if r > 0 else 0

    with tile.TileContext(nc) as tc:
        with contextlib.ExitStack() as ctx:
            persist = ctx.enter_context(tc.tile_pool(name="persist", bufs=1))
            dram = ctx.enter_context(tc.tile_pool(name="dram", bufs=1,
                                                  space="DRAM"))

            ones_col_bf = persist.tile([128, 1], BF16, name="ones_col_bf")
            nc.vector.memset(ones_col_bf[:], 1.0)
            ones_row_bf = persist.tile([1, 128], BF16, name="ones_row_bf")
            nc.vector.memset(ones_row_bf[:], 1.0)

            # tiny exp at t=0 preloads the ACT function table
            dum_exp = persist.tile([1, 16], F32, name="dum_exp")
            nc.vector.memset(dum_exp[:], 0.0)
            nc.scalar.activation(dum_exp[:], dum_exp[:], AF.Exp)

            # resident Q^T / K^T / V per batch (bf16)
            q_res = [[persist.tile([128, SQ], BF16, name=f"q{h}{b}")
                      for b in range(B)] for h in range(2)]
            k_res = [persist.tile([128, SQ], BF16, name=f"k{b}")
                     for b in range(B)]
            v_res = [persist.tile([128, 16, 128], BF16, name=f"v{b}")
                     for b in range(B)]
            g_all = [[persist.tile([128, NCORES, 256], BF16, name=f"g{b}{h}")
                      for h in range(2)] for b in range(B)]

            # A2A bounce buffers, one per (b, h_local) chunk
            cc_in = [[dram.tile([NCORES * 128, 256], BF16, name=f"ccin{b}{h}")
                      for h in range(2)] for b in range(B)]
            cc_out = [[dram.tile([NCORES, 128, 256], BF16, name=f"ccout{b}{h}")
                       for h in range(2)] for b in range(B)]
            # pair K/V exchange bounce buffers, per batch
            kv_in = [dram.tile([128, SQ], BF16, name=f"kvin{b}")
                     for b in range(B)]
            kv_out = [dram.tile([2, 128, SQ], BF16, name=f"kvout{b}")
                      for b in range(B)]
            # long-lived attention SBUF pools (below the proj pools on the
            # stack so proj pools can close mid-attention)
            p2 = ctx.enter_context(tc.tile_pool(name="p2", bufs=1))
            p2s = ctx.enter_context(tc.tile_pool(name="p2s", bufs=2))
            p2c = ctx.enter_context(tc.tile_pool(name="p2c", bufs=2))
            p3o = ctx.enter_context(tc.tile_pool(name="p3o", bufs=2))
            # PSUM: acc tag = proj streams + dense-b0 groups (2 banks)
            accps = ctx.enter_context(tc.tile_pool(name="accps", bufs=2,
                                                   space="PSUM"))

            # ---- projection: weights + x^T staged via rotating pool ----
            proj_ctx = contextlib.ExitStack()
            p1w = proj_ctx.enter_context(tc.tile_pool(name="p1w", bufs=1))
            p1x = proj_ctx.enter_context(tc.tile_pool(name="p1x", bufs=4))
            p1 = proj_ctx.enter_context(tc.tile_pool(name="p1", bufs=2))

            wq_sb = p1w.tile([128, 16, 256], BF16, name="wq_sb")
            wkv_sb = p1w.tile([128, 16, 128], BF16, name="wkv_sb")
            # x^T: ONE tile per 512-token chunk (few descriptors — the
            # queues retire only ~1/0.64us). One DMA ring sustains only
            # ~150GB/s, so the startup-critical loads (wkv + x^T sc=0)
            # are split across the sync/vector/gpsimd/scalar rings; the
            # steady-state sc=1..7 stream stays on scalar (slot-waits may
            # park it, and scalar has nothing urgent until the exps).
            # sc=0 + wkv split into ~0.5MB pieces across 3 rings, ordered
            # so kv-group matmuls (which consume hc 0..15 in order) start
            # after the first piece; subtile deps release per-slice
            nc.sync.dma_start(out=wkv_sb[:, 0:8, :], in_=wkv.ap()[:, 0:1024])
            xts = []
            for sc in range(NTC):
                tsb = p1x.tile([128, 16 * TC], BF16, tag="xts", bufs=4,
                               name=f"xts{sc}")
                if sc == 0:
                    Q = 4 * TC
                    nc.sync.dma_start(out=tsb[:, 0:Q],
                                      in_=xt.ap()[:, 0:Q])
                    nc.gpsimd.dma_start(out=tsb[:, Q:2 * Q],
                                        in_=xt.ap()[:, Q:2 * Q])
                    nc.gpsimd.dma_start(out=wkv_sb[:, 8:16, :],
                                        in_=wkv.ap()[:, 1024:2048])
                    nc.scalar.dma_start(out=tsb[:, 2 * Q:3 * Q],
                                        in_=xt.ap()[:, 2 * Q:3 * Q])
                    nc.scalar.dma_start(out=tsb[:, 3 * Q:4 * Q],
                                        in_=xt.ap()[:, 3 * Q:4 * Q])
                    nc.sync.dma_start(out=wq_sb[:], in_=wq.ap())
                else:
                    nc.scalar.dma_start(out=tsb[:],
                                        in_=xt.ap()[:, ts(sc, 16 * TC)])
                xts.append(tsb)

            W_OF = {"kv": lambda hc: wkv_sb[:, hc, :],
                    "q0": lambda hc: wq_sb[:, hc, 0:128],
                    "q1": lambda hc: wq_sb[:, hc, 128:256]}

            def make_proj_group(tci, which):
                b, sc = tci // 4, tci % 4

                def thunk():
                    ps = accps.tile([128, TC], F32, tag="acc",
                                    name=f"ps_{which}{tci}")
                    wof = W_OF[which]
                    for hc in range(16):
                        nc.tensor.matmul(ps[:], wof(hc),
                                         xts[tci][:, ds(hc * TC, TC)],
                                         start=(hc == 0), stop=(hc == 15),
                                         skip_group_check=True)
                    if which == "kv":
                        kvt = p1.tile([128, TC], BF16, tag="kvt", name="kvt")
                        nc.vector.tensor_copy(kvt[:], ps[:])
                        nc.sync.dma_start(out=kv_in[b][:, ts(sc, TC)],
                                          in_=kvt[:])
                    elif which == "q0":
                        nc.vector.tensor_copy(q_res[0][b][:, ts(sc, TC)],
                                              ps[:])
                    else:
                        nc.vector.tensor_copy(q_res[1][b][:, ts(sc, TC)],
                                              ps[:])
                return thunk

            def emit_kv_exchange(b):
                # pair AllGather (pair-rank 0 = K), then K^T/V resident loads
                nc.gpsimd.collective_compute(
                    "AllGather", mybir.AluOpType.bypass,
                    replica_groups=PAIRS,
                    ins=[kv_in[b].opt()], outs=[kv_out[b].opt()])
                for kc in range(4):
                    nc.sync.dma_start(out=k_res[b][:, ts(kc, TC)],
                                      in_=kv_out[b][0, :, ts(kc, TC)])
                for s4 in range(4):
                    nc.sync.dma_start(out=v_res[b][:, ds(4 * s4, 4), :],
                                      in_=kv_out[b][1, :, ts(s4, TC)],
                                      transpose=True)

            # tci 0..3 in-line, then ALL b=1 kv groups + the b=1 exchange
            # in-line (so its AllGather fires ~75us, not behind chunk
            # (0,0)'s GpSimd stream), then q groups of tci 4..5; q groups
            # of tci 6..7 become fill work zipped into attention chunk
            # (0,0)
            for tci in range(4):
                for which in ("kv", "q0", "q1"):
                    make_proj_group(tci, which)()
                if tci == 3:
                    emit_kv_exchange(0)
            for tci in range(4, 8):
                make_proj_group(tci, "kv")()
            emit_kv_exchange(1)
            for tci in (4, 5):
                for which in ("q0", "q1"):
                    make_proj_group(tci, which)()
            proj_fills = [make_proj_group(tci, which)
                          for tci in (6, 7)
                          for which in ("q0", "q1")]

            # wd (8MB) + dense staging allocated later, after proj pools free
            wd_holder = {}

            def make_dense_b0(u, oc):
                def thunk():
                    wd_sb = wd_holder["wd"]
                    ps = accps.tile([128, TC], F32, tag="acc",
                                    name=f"d0_{u}{oc}")
                    for ec in range(16):
                        hl, i = ec // 8, ec % 8
                        nc.tensor.matmul(
                            ps[:], g_all[0][hl][:, i, ts(u, 128)],
                            wd_sb[:, 2 * i + hl, ts(oc, 512)],
                            start=(ec == 0), stop=(ec == 15),
                            skip_group_check=True)
                    osb = p3o.tile([128, 512], F32, tag="osb", name="osb")
                    nc.vector.tensor_copy(osb[:], ps[:])
                    nc.sync.dma_start(
                        out=out.ap()[ds(u * 128, 128), ts(oc, 512)],
                        in_=osb[:])
                return thunk

            dense_b0 = [make_dense_b0(u, oc) for u in range(2)
                        for oc in range(4)]

            # ---- attention per (b, h_local), scores^T [k, q] ----
            # ScalarE's exp gates the score stream (2-slot score-PSUM
            # rotation); PV/fold/broadcast matmuls + external fill groups
            # (proj / dense-b0) are zipped between score matmuls, with each
            # chunk's qg2/qg3 tail deferred into the NEXT chunk's stream.
            CHUNKS = [(0, 0), (0, 1), (1, 0), (1, 1)]
            # full-width (off=0) tiles summed on GpSimd; diagonal + head on DVE
            GPS_SPLIT = {0: [], 1: [2, 3], 2: [2, 3, 4, 5, 6, 7],
                         3: [4, 5, 6, 7, 8, 9]}
            # ext fill-group pop schedule per chunk index.  dense-b0
            # groups contract over BOTH hl gathers, so none can run
            # before chunk (1,1) (g_all[0][1] flushes at its start).
            EXT_FILLS = {0: proj_fills,
                         3: dense_b0[0:4]}

            att_ctx = contextlib.ExitStack()
            attps = att_ctx.enter_context(
                tc.tile_pool(name="attps", bufs=2, space="PSUM"))

            prev_core, prev_bc3 = [], None
            pending_gathers = []

            def flush_gathers():
                # emitted >=1 chunk after the A2A trigger so the Sync
                # queue never parks long on an in-flight collective
                while pending_gathers:
                    gb, gh = pending_gathers.pop(0)
                    nc.sync.dma_start(
                        out=g_all[gb][gh][:],
                        in_=cc_out[gb][gh].rearrange("i p s -> p i s"))

            for ci, (b, hl) in enumerate(CHUNKS):
                flush_gathers()
                q_src = q_res[hl][b]
                st = {}
                for qg in range(4):
                    st[("et", qg)] = p2.tile([128, 4 * (qg + 1) * 512],
                                             BF16, tag=f"et{qg}", bufs=1,
                                             name=f"et{qg}")

                def emit_sc_one(qg, kt, st=st, b=b, q_src=q_src):
                    slab = st[("et", qg)]
                    o = off_of(kt, qg)
                    w = 512 - o
                    sc_ps = attps.tile([128, 1024], F32, tag="sc",
                                       name="scs")
                    nc.tensor.matmul(
                        sc_ps[:, 0:w], k_res[b][:, ts(kt, 128)],
                        q_src[:, ds(qg * 512 + o, w)],
                        start=True, stop=True, skip_group_check=True)
                    nc.scalar.activation(
                        slab[:, ds(512 * kt + o, w)], sc_ps[:, 0:w],
                        AF.Exp)
                    if kt - 4 * qg >= 0:
                        # zero E^T where q < k inside the 128-wide
                        # diagonal block (beyond it q >= k always)
                        nc.gpsimd.affine_select(
                            out=slab[:, ds(512 * kt + o, 128)],
                            in_=slab[:, ds(512 * kt + o, 128)],
                            compare_op=mybir.AluOpType.is_ge,
                            fill=0.0, base=0,
                            pattern=[[1, 128]],
                            channel_multiplier=-1)

                def emit_sc_pair(qg, kt, st=st, b=b, q_src=q_src):
                    # two full k-tiles share one 2-bank score PSUM and
                    # one flat [128,1024] exp (halves the ACT per-op
                    # overhead)
                    slab = st[("et", qg)]
                    sc_ps = attps.tile([128, 1024], F32, tag="sc",
                                       name="scp")
                    for j in range(2):
                        nc.tensor.matmul(
                            sc_ps[:, ts(j, 512)],
                            k_res[b][:, ts(kt + j, 128)],
                            q_src[:, ts(qg, 512)],
                            start=True, stop=True, skip_group_check=True)
                    nc.scalar.activation(
                        slab[:, ds(512 * kt, 1024)], sc_ps[:, 0:1024],
                        AF.Exp)

                def emit_chain(qg, st=st):
                    nkt = 4 * (qg + 1)
                    slab = st[("et", qg)]
                    gps = GPS_SPLIT[qg]
                    dve = [kt for kt in range(nkt) if kt not in gps]
                    padd = p2s.tile([128, 512], BF16, tag=f"padd{qg}",
                                    name=f"padd{qg}")
                    if qg == 0:
                        nc.vector.tensor_copy(padd[:],
                                              slab[:, ds(0, 512)])
                        rest = dve[1:]
                    else:
                        nc.vector.tensor_add(padd[:],
                                             slab[:, ds(512 * dve[0], 512)],
                                             slab[:, ds(512 * dve[1], 512)])
                        rest = dve[2:]
                    for kt in rest:
                        o = off_of(kt, qg)
                        nc.vector.tensor_add(
                            padd[:, o:512], padd[:, o:512],
                            slab[:, ds(512 * kt + o, 512 - o)])
                    if gps:
                        padd_g = p2s.tile([128, 512], BF16,
                                          tag=f"paddg{qg}",
                                          name=f"paddg{qg}")
                        nc.gpsimd.tensor_add(padd_g[:],
                                             slab[:, ds(512 * gps[0], 512)],
                                             slab[:, ds(512 * gps[1], 512)])
                        for kt in gps[2:]:
                            nc.gpsimd.tensor_add(padd_g[:], padd_g[:],
                                                 slab[:, ds(512 * kt, 512)])
                        nc.vector.tensor_add(padd[:], padd[:], padd_g[:])
                    st[("padd", qg)] = padd

                def make_pv(qg, kt, st=st, b=b):
                    nkt = 4 * (qg + 1)

                    def thunk():
                        if kt == 0:
                            st[("ctx", qg)] = attps.tile(
                                [128, 512], F32, tag="ctx",
                                name=f"ctx{qg}")
                        o = off_of(kt, qg)
                        nc.tensor.matmul(
                            st[("ctx", qg)][:, o:512],
                            v_res[b][:, kt, :],
                            st[("et", qg)][:, ds(512 * kt + o, 512 - o)],
                            start=(kt == 0), stop=(kt == nkt - 1),
                            skip_group_check=True)
                    return thunk

                def make_fold(qg, st=st):
                    def thunk():
                        padd = st[("padd", qg)]
                        rs_ps = accps.tile([1, 512], F32, tag="acc",
                                           name="rs")
                        nc.tensor.matmul(rs_ps[:], ones_col_bf[:], padd[:],
                                         start=True, stop=True,
                                         skip_group_check=True)
                        rs_bf = p2s.tile([1, 512], BF16, tag="rsbf",
                                         name="rsbf")
                        nc.vector.tensor_copy(rs_bf[:], rs_ps[:])
                        st[("rsbf", qg)] = rs_bf
                    return thunk

                def make_bc(qg, st=st, b=b, hl=hl, last=False):
                    def thunk():
                        bc_ps = accps.tile([128, 512], F32, tag="acc",
                                           name="bc")
                        nc.tensor.matmul(bc_ps[:], ones_row_bf[:],
                                         st[("rsbf", qg)][:],
                                         start=True, stop=True,
                                         skip_group_check=True)
                        rinv = p2s.tile([128, 512], F32, tag="rinv",
                                        name="rinv")
                        nc.vector.reciprocal_approx_fast(rinv[:], bc_ps[:])
                        ctxt = p2c.tile([128, 512], BF16, tag="ctxt",
                                        name="ctxt")
                        nc.vector.tensor_mul(ctxt[:], st[("ctx", qg)][:],
                                             rinv[:])
                        for half in range(2):
                            peer = 2 * qg + half
                            nc.sync.dma_start(
                                out=cc_in[b][hl][ts(peer, 128), :],
                                in_=ctxt[:, ts(half, 256)])
                        if last:
                            nc.gpsimd.collective_compute(
                                "AllToAll", mybir.AluOpType.bypass,
                                replica_groups=[list(range(NCORES))],
                                ins=[cc_in[b][hl].opt()],
                                outs=[cc_out[b][hl].opt()])
                            pending_gathers.append((b, hl))
                    return thunk

                P = {qg: [make_pv(qg, kt) for kt in range(4 * (qg + 1))]
                     for qg in range(4)}
                F = {qg: make_fold(qg) for qg in range(4)}
                BC = {qg: make_bc(qg, last=(qg == 3)) for qg in range(4)}

                # fill order staggers folds/broadcasts away from their
                # DVE dependencies (fold after its chain, bc ~recip-lag
                # after its fold, muls between consecutive bc's)
                if ci == 0:
                    fills = [*P[0], F[0], P[1][0], P[1][1], BC[0],
                             *P[1][2:], F[1], BC[1]]
                else:
                    fills = list(prev_core) + list(P[0])
                    fills.append(prev_bc3)
                    fills += [F[0], P[1][0], P[1][1], BC[0], *P[1][2:],
                              F[1], BC[1]]
                prev_core = [*P[2], F[2], P[3][0], P[3][1], BC[2],
                             *P[3][2:], F[3]]
                prev_bc3 = BC[3]

                units = []
                for qg in range(4):
                    nfull = 4 * qg + 1
                    kt = 0
                    while kt + 1 < nfull:
                        units.append((qg, kt, 2))
                        kt += 2
                    while kt < 4 * (qg + 1):
                        units.append((qg, kt, 1))
                        kt += 1

                # greedy pacing: keep cumulative emitted PE time (scores +
                # fills) ~margin ahead of cumulative exp time, so the
                # score stream never out-runs ScalarE into a PE stall and
                # fills don't bunch at the chunk front.  Small fills
                # (PV/fold/bc ~0.26us) pop first; 16-MM ext groups (4.2us)
                # pop when the deficit is large.
                ext = list(EXT_FILLS.get(ci, []))
                fi = 0
                MM = 270            # ns, one 512-wide matmul at 13/16 clk
                GRP = 16 * MM
                pe_t, exp_t = 0.0, 0.0
                for ui, (qg, kt, npair) in enumerate(units):
                    if npair == 2:
                        emit_sc_pair(qg, kt)
                        exp_t += (1024 + 352) / 1.2
                    else:
                        o = off_of(kt, qg)
                        emit_sc_one(qg, kt)
                        exp_t += (512 - o + 352) / 1.2
                    pe_t += npair * MM
                    # chunk (0,0): own PV fills are only ready ~2 units
                    # after their exp, so the always-ready proj groups
                    # pop first; later chunks start with ready deferred
                    # fills and keep ext groups for the deficit tail
                    while pe_t < exp_t + 1500:
                        if ci == 0 and ext:
                            ext.pop(0)()
                            pe_t += GRP
                        elif fi < len(fills):
                            fills[fi]()
                            fi += 1
                            pe_t += MM
                        elif ext:
                            ext.pop(0)()
                            pe_t += GRP
                        else:
                            break
                    if kt + npair == 4 * (qg + 1):
                        emit_chain(qg)
                while fi < len(fills):
                    fills[fi]()
                    fi += 1
                for t in ext:
                    t()

                if ci == 0:
                    # proj inputs fully consumed: release wq/wkv/x^T SBUF,
                    # then load dense weights into the freed space
                    proj_ctx.close()
                    wdp = ctx.enter_context(tc.tile_pool(name="wdp",
                                                         bufs=1))
                    wd_sb = wdp.tile([128, 16, HIDDEN], BF16, name="wd_sb")
                    nc.sync.dma_start(out=wd_sb[:], in_=wd.ap())
                    wd_holder["wd"] = wd_sb

            # tail of the last chunk, then its A2A + the dense remainder
            for t in prev_core:
                t()
            prev_bc3()
            flush_gathers()
            for t in dense_b0[4:8]:
                t()
            att_ctx.close()

            # ---- dense b=1: hl-split halves so the hl=0 half runs while
            # the last A2A drains; 6-deep PSUM rotation ----
            wd_sb = wd_holder["wd"]
            with tc.tile_pool(name="dnps", bufs=6, space="PSUM") as dnps:
                ops_tiles = {}

                def d1_h0(u, oc):
                    ps = dnps.tile([128, 512], F32, tag="ops",
                                   name=f"d1_{u}{oc}")
                    ops_tiles[(u, oc)] = ps
                    for i in range(8):
                        nc.tensor.matmul(
                            ps[:], g_all[1][0][:, i, ts(u, 128)],
                            wd_sb[:, 2 * i, ts(oc, 512)],
                            start=(i == 0), stop=False,
                            skip_group_check=True)

                def d1_h1(u, oc):
                    ps = ops_tiles[(u, oc)]
                    for i in range(8):
                        nc.tensor.matmul(
                            ps[:], g_all[1][1][:, i, ts(u, 128)],
                            wd_sb[:, 2 * i + 1, ts(oc, 512)],
                            start=False, stop=(i == 7),
                            skip_group_check=True)
                    osb = p3o.tile([128, 512], F32, tag="osb", name="osb1")
                    nc.vector.tensor_copy(osb[:], ps[:])
                    nc.sync.dma_start(
                        out=out.ap()[ds(256 + u * 128, 128), ts(oc, 512)],
                        in_=osb[:])

                d1_h0(0, 0); d1_h0(0, 1); d1_h0(0, 2); d1_h0(0, 3)
                d1_h0(1, 0); d1_h0(1, 1)
                d1_h1(0, 0); d1_h0(1, 2)
                d1_h1(0, 1); d1_h0(1, 3)
                d1_h1(0, 2); d1_h1(0, 3)
                d1_h1(1, 0); d1_h1(1, 1); d1_h1(1, 2); d1_h1(1, 3)

    nc.compile()
    return nc


def kernel(x, w_q, w_kv, w_dense):
    from concourse.bass_utils import run_bass_kernel_spmd

    bf16 = ml_dtypes.bfloat16
    x = np.asarray(x, dtype=np.float32)
    w_q = np.asarray(w_q, dtype=np.float32)
    w_kv = np.asarray(w_kv, dtype=np.float32)
    w_dense = np.asarray(w_dense, dtype=np.float32)

    # partition-major x^T: xt[p, sc, hc, t] = x[sc*512+t, hc*128+p]
    xt = np.ascontiguousarray(
        x.reshape(NTC, TC, 16, 128).transpose(3, 0, 2, 1).reshape(128, -1)
    ).astype(bf16)
    wq_s = (w_q * SCALE).astype(bf16)          # fold softmax scale into Q proj
    wkv_b = w_kv.astype(bf16)
    wd_b = w_dense.astype(bf16)

    def pretile(w):
        # [2048, e] -> SBUF layout [p, hc*e]: row p, col hc*e_sz + e
        e_sz = w.shape[1]
        return np.ascontiguousarray(
            w.reshape(16, 128, e_sz).transpose(1, 0, 2).reshape(128, 16 * e_sz))

    wd_t = pretile(wd_b)
    in_maps = []
    for c in range(NCORES):
        g = c // 2
        if c % 2 == 0:
            wkv_c = wkv_b[:, 128 * g:128 * (g + 1)]                # K half
        else:
            wkv_c = wkv_b[:, 512 + 128 * g:512 + 128 * (g + 1)]    # V half
        in_maps.append({
            "xt": xt,
            "wq": pretile(wq_s[:, 256 * c:256 * (c + 1)]),
            "wkv": pretile(wkv_c),
            "wd": wd_t,
        })

    if "nc" not in _CACHE:
        _CACHE["nc"] = _build()
    nc = _CACHE["nc"]

    res = run_bass_kernel_spmd(nc, in_maps, core_ids=list(range(NCORES)))
    kernel.last_results = res
    kernel.last_exec_time_ns = res.exec_time_ns

    out_full = np.empty((T, HIDDEN), dtype=np.float32)
    for c in range(NCORES):
        r = res.results[c]["out"]              # [512, 2048]
        for b in range(B):
            out_full[b * SQ + 256 * c: b * SQ + 256 * (c + 1), :] = \
                r[b * 256:(b + 1) * 256, :]
    return out_full.reshape(B, SQ, HIDDEN)
